# revision 9
# baseline (speedup 1.0000x reference)
"""ALIF (adaptive leaky integrate-and-fire) spiking-neuron scan on 8 TRN2 cores.

Problem: x_seq [T=1024, B=32, N=2048] f32; per-neuron recurrence over T:
    v  = decay_v*v + x_t
    th = threshold + beta*a
    s  = (v - th > 0)
    v  = v - s*th
    a  = decay_a*a + s
Output: spikes [T, B, N] f32.

Sharding: flatten neurons (B*N = 65536), shard 8192 neurons per core across
8 cores; each core scans its slice independently (no collectives).

Kernel strategy (per core):
  - neurons laid out [128 partitions x 64 free]; time is sequential.
  - per step, 3 custom DVE instructions:
      w  = decay_v*v + x_t                     (stock AFFINE_THEN_ADD)
      a' = decay_a*a + (w - (beta*a+th) > 0)   (ALIF_ASPIKE, writes a-trajectory)
      v' = select(w-(beta*a+th) > 0, w-(beta*a+th), w)   (ALIF_RESET)
  - spikes are recovered in bulk per chunk from the a-trajectory:
      s_t = (a_{t+1} != decay_a*a_t)           (ALIF_SRECOV)
    (exact: a' = round(round(la*a)+s), and +1 is always exact at these magnitudes)
  - x loads / s stores are strided chunk DMAs (64 timesteps per chunk).

All arithmetic matches the reference's fp32 op order exactly (DVE rounds fp32
per ALU stage, same as numpy op-by-op), so spikes match bit-for-bit modulo
FMA-vs-separate rounding in the jax CPU reference.
"""

import numpy as np

from concourse import bacc, bass, mybir
from concourse.bass_utils import run_bass_kernel_spmd
from concourse.tile import TileContext
import concourse.dve_ops as dve_ops_mod
from concourse.dve_ops import AFFINE_THEN_ADD
from concourse.dve_spec import (
    Spec,
    Src0,
    Src1,
    C0,
    C1,
    C2,
    Zero,
    One,
    select,
    ne,
    lower,
    _has_src1 as has_src1,
)
from concourse.dve_uop import DveOpSpec

# ---------------------------------------------------------------- constants
T = 1024
N_FLAT = 65536
N_CORES = 8
NCORE = N_FLAT // N_CORES  # 8192 neurons per core
P = 128
F = NCORE // P  # 64 neurons per partition
TC = 64  # timesteps per chunk
NCHUNK = T // TC
FD = TC * F  # free-dim elements per x/s chunk tile


# ------------------------------------------------------- custom DVE ops
def _register(name: str, spec: Spec, subdim: bool = False):
    """Register a custom DVE op into the module-level catalog at runtime,
    computing its uops_sha pins so DveOp.compile()'s drift check passes."""
    for op in dve_ops_mod.OPS:
        if op.name == name:
            return op
    row = dve_ops_mod._CUSTOM_DVE_ROW_BASE + len(dve_ops_mod.OPS)
    assert row < 0x20, "DVE opcode-table row space exhausted"
    shas = {}
    for ver in ("v3", "v4"):
        uops = lower(spec, ver=ver)
        shas[ver] = DveOpSpec(
            name=name, opcode=row, uops=uops, rd1_en=has_src1(spec)
        ).sha(ver)
    op = dve_ops_mod.DveOp(name, spec, subdim, shas)
    dve_ops_mod.OPS.append(op)
    dve_ops_mod.CUSTOM_DVE_SPECS[name] = spec
    dve_ops_mod._SUB_OPCODE_FOR_NAME[name] = row
    return op


# v' = select(d > 0, d, w)  with d = w - (a*beta + th);  in0=w, in1=a
_d_reset = Src0 - (Src1 * C1 + C0)
ALIF_RESET = _register(
    "ALIF_RESET",
    Spec(
        body=select(_d_reset > Zero, _d_reset, Src0),
        reference=lambda in0, in1, s0, s1, imm2: np.where(
            (in0 - (in1 * s1 + s0)) > 0, in0 - (in1 * s1 + s0), in0
        ).astype(np.float32),
    ),
)

# a' = la*a + (w > beta*a+th ? 1 : 0);  in0=a, in1=w.
# (w > th) == (round(w-th) > 0) exactly: near ties Sterbenz makes w-th exact.
_th_aspk = Src0 * C1 + C0
_m_aspk = Src0 * C2
ALIF_ASPIKE = _register(
    "ALIF_ASPIKE",
    Spec(
        body=select(Src1 > _th_aspk, _m_aspk + One, _m_aspk),
        reference=lambda in0, in1, s0, s1, imm2: np.where(
            in1 > (in0 * s1 + s0),
            (in0 * imm2).astype(np.float32) + np.float32(1.0),
            (in0 * imm2).astype(np.float32),
        ).astype(np.float32),
    ),
)

# s_t = (a_{t+1} != la*a_t);  in0=a[t+1] stream, in1=a[t] stream
ALIF_SRECOV = _register(
    "ALIF_SRECOV",
    Spec(
        body=ne(Src0, Src1 * C0),
        reference=lambda in0, in1, s0, s1, imm2: (
            in0 != (in1 * s0).astype(np.float32)
        ).astype(np.float32),
    ),
)


# ------------------------------------------------------------- graph build
def build_graph(lam_v: float, lam_a: float, theta: float, beta: float) -> bass.Bass:
    f32 = mybir.dt.float32
    nc = bacc.Bacc(
        "TRN2",
        target_bir_lowering=False,
        debug=False,
        num_devices=N_CORES,
    )

    x_ext = nc.dram_tensor("x", [T, NCORE], f32, kind="ExternalInput").ap()
    v_ext = nc.dram_tensor("v", [P, F], f32, kind="ExternalInput").ap()
    a_ext = nc.dram_tensor("a", [P, F], f32, kind="ExternalInput").ap()
    out_ext = nc.dram_tensor("out", [T, NCORE], f32, kind="ExternalOutput").ap()

    with TileContext(nc) as tc:
        with (
            tc.tile_pool(name="state", bufs=1) as state_pool,
            tc.tile_pool(name="xs", bufs=3) as x_pool,
            tc.tile_pool(name="ss", bufs=3) as s_pool,
            tc.tile_pool(name="at", bufs=2) as a_pool,
            tc.tile_pool(name="wt", bufs=2) as w_pool,
        ):
            v = state_pool.tile([P, F], f32)
            nc.sync.dma_start(out=v[:], in_=v_ext[:])

            prev_atraj = None
            for k in range(NCHUNK):
                xt = x_pool.tile([P, FD], f32, tag="x")
                nc.sync.dma_start(
                    out=xt[:].rearrange("p (t f) -> p t f", f=F),
                    in_=x_ext[k * TC : (k + 1) * TC, :].rearrange(
                        "t (p f) -> p t f", p=P
                    ),
                )

                atraj = a_pool.tile([P, (TC + 1) * F], f32, tag="atraj")
                if k == 0:
                    nc.sync.dma_start(out=atraj[:, 0:F], in_=a_ext[:])
                else:
                    nc.vector.tensor_copy(
                        atraj[:, 0:F], prev_atraj[:, TC * F : (TC + 1) * F]
                    )

                for t in range(TC):
                    w = w_pool.tile([P, F], f32, tag="w")
                    a_prev = atraj[:, t * F : (t + 1) * F]
                    x_t = xt[:, t * F : (t + 1) * F]
                    nc.vector._custom_dve(
                        AFFINE_THEN_ADD, out=w[:], in0=v[:], in1=x_t,
                        s0=lam_v, s1=0.0,
                    )
                    nc.vector._custom_dve(
                        ALIF_ASPIKE,
                        out=atraj[:, (t + 1) * F : (t + 2) * F],
                        in0=a_prev, in1=w[:],
                        s0=theta, s1=beta, imm2=lam_a,
                    )
                    nc.vector._custom_dve(
                        ALIF_RESET, out=v[:], in0=w[:], in1=a_prev,
                        s0=theta, s1=beta,
                    )

                st = s_pool.tile([P, FD], f32, tag="s")
                # spike recovery on GPSIMD (concurrent with the DVE chain):
                # d = a_{t+1} - la*a_t is exactly 0 or 1+eps; min(2d,1) is
                # exactly the spike bit.
                ad = s_pool.tile([P, FD], f32, tag="ad")
                nc.gpsimd.tensor_scalar_mul(ad[:], atraj[:, 0 : TC * F], lam_a)
                nc.gpsimd.tensor_sub(ad[:], atraj[:, F : (TC + 1) * F], ad[:])
                nc.gpsimd.tensor_scalar(
                    st[:], ad[:], 2.0, 1.0,
                    mybir.AluOpType.mult, mybir.AluOpType.min,
                )
                nc.sync.dma_start(
                    out=out_ext[k * TC : (k + 1) * TC, :].rearrange(
                        "t (p f) -> p t f", p=P
                    ),
                    in_=st[:].rearrange("p (t f) -> p t f", f=F),
                )
                prev_atraj = atraj

    nc.compile()
    return nc


_GRAPH_CACHE: dict[tuple, bass.Bass] = {}


def _get_graph(lam_v, lam_a, theta, beta):
    key = (lam_v, lam_a, theta, beta)
    if key not in _GRAPH_CACHE:
        _GRAPH_CACHE[key] = build_graph(lam_v, lam_a, theta, beta)
    return _GRAPH_CACHE[key]


# ------------------------------------------------------------------ entry
def kernel(
    x_seq, v, a, decay_v, decay_a, threshold, beta, alpha, _want_trace=False
):
    x = np.ascontiguousarray(np.asarray(x_seq, dtype=np.float32))
    orig_shape = x.shape
    x2 = x.reshape(T, N_FLAT)
    v = np.asarray(v, dtype=np.float32).reshape(N_FLAT)
    a = np.asarray(a, dtype=np.float32).reshape(N_FLAT)

    lam_v = float(np.float32(decay_v))
    lam_a = float(np.float32(decay_a))
    theta = float(np.float32(threshold))
    beta_f = float(np.float32(beta))

    nc = _get_graph(lam_v, lam_a, theta, beta_f)

    in_maps = []
    for c in range(N_CORES):
        lo, hi = c * NCORE, (c + 1) * NCORE
        in_maps.append(
            {
                "x": np.ascontiguousarray(x2[:, lo:hi]),
                "v": np.ascontiguousarray(v[lo:hi].reshape(P, F)),
                "a": np.ascontiguousarray(a[lo:hi].reshape(P, F)),
            }
        )

    res = run_bass_kernel_spmd(
        nc, in_maps, core_ids=list(range(N_CORES)), trace=_want_trace
    )
    outs = [res.results[c]["out"] for c in range(N_CORES)]
    full = np.concatenate(outs, axis=1).reshape(orig_shape)
    if _want_trace:
        return full, res
    return full


# revision 10
# speedup vs baseline: 4.1118x; 4.1118x over previous
"""ALIF (adaptive leaky integrate-and-fire) spiking-neuron scan on 8 TRN2 cores.

Problem: x_seq [T=1024, B=32, N=2048] f32; per-neuron recurrence over T:
    v  = decay_v*v + x_t
    th = threshold + beta*a
    s  = (v - th > 0)
    v  = v - s*th
    a  = decay_a*a + s
Output: spikes [T, B, N] f32.

Sharding: flatten neurons (B*N = 65536), shard 8192 neurons per core across
8 cores; each core scans its slice independently (no collectives).

Kernel strategy (per core):
  - neurons laid out [128 partitions x 64 free]; time is sequential.
  - per step, 3 custom DVE instructions:
      w  = decay_v*v + x_t                     (stock AFFINE_THEN_ADD)
      a' = decay_a*a + (w - (beta*a+th) > 0)   (ALIF_ASPIKE, writes a-trajectory)
      v' = select(w-(beta*a+th) > 0, w-(beta*a+th), w)   (ALIF_RESET)
  - spikes are recovered in bulk per chunk from the a-trajectory:
      s_t = (a_{t+1} != decay_a*a_t)           (ALIF_SRECOV)
    (exact: a' = round(round(la*a)+s), and +1 is always exact at these magnitudes)
  - x loads / s stores are strided chunk DMAs (64 timesteps per chunk).

All arithmetic matches the reference's fp32 op order exactly (DVE rounds fp32
per ALU stage, same as numpy op-by-op), so spikes match bit-for-bit modulo
FMA-vs-separate rounding in the jax CPU reference.
"""

import numpy as np

from concourse import bacc, bass, mybir
from concourse.bass_utils import run_bass_kernel_spmd
from concourse.tile import TileContext
import concourse.dve_ops as dve_ops_mod
from concourse.dve_ops import AFFINE_THEN_ADD
from concourse.dve_spec import (
    Spec,
    Src0,
    Src1,
    C0,
    C1,
    C2,
    Zero,
    One,
    select,
    ne,
    lower,
    _has_src1 as has_src1,
)
from concourse.dve_uop import DveOpSpec

# ---------------------------------------------------------------- constants
T = 1024
N_FLAT = 65536
N_CORES = 8
NCORE = N_FLAT // N_CORES  # 8192 neurons per core
P = 128
F = NCORE // P  # 64 neurons per partition
TC = 64  # timesteps per chunk
NCHUNK = T // TC
FD = TC * F  # free-dim elements per x/s chunk tile


# ------------------------------------------------------- custom DVE ops
def _register(name: str, spec: Spec, subdim: bool = False):
    """Register a custom DVE op into the module-level catalog at runtime,
    computing its uops_sha pins so DveOp.compile()'s drift check passes."""
    for op in dve_ops_mod.OPS:
        if op.name == name:
            return op
    row = dve_ops_mod._CUSTOM_DVE_ROW_BASE + len(dve_ops_mod.OPS)
    assert row < 0x20, "DVE opcode-table row space exhausted"
    shas = {}
    for ver in ("v3", "v4"):
        uops = lower(spec, ver=ver)
        shas[ver] = DveOpSpec(
            name=name, opcode=row, uops=uops, rd1_en=has_src1(spec)
        ).sha(ver)
    op = dve_ops_mod.DveOp(name, spec, subdim, shas)
    dve_ops_mod.OPS.append(op)
    dve_ops_mod.CUSTOM_DVE_SPECS[name] = spec
    dve_ops_mod._SUB_OPCODE_FOR_NAME[name] = row
    return op


# v' = select(d > 0, d, w)  with d = w - (a*beta + th);  in0=w, in1=a
_d_reset = Src0 - (Src1 * C1 + C0)
ALIF_RESET = _register(
    "ALIF_RESET",
    Spec(
        body=select(_d_reset > Zero, _d_reset, Src0),
        reference=lambda in0, in1, s0, s1, imm2: np.where(
            (in0 - (in1 * s1 + s0)) > 0, in0 - (in1 * s1 + s0), in0
        ).astype(np.float32),
    ),
)

# a' = la*a + (w > beta*a+th ? 1 : 0);  in0=a, in1=w.
# (w > th) == (round(w-th) > 0) exactly: near ties Sterbenz makes w-th exact.
_th_aspk = Src0 * C1 + C0
_m_aspk = Src0 * C2
ALIF_ASPIKE = _register(
    "ALIF_ASPIKE",
    Spec(
        body=select(Src1 > _th_aspk, _m_aspk + One, _m_aspk),
        reference=lambda in0, in1, s0, s1, imm2: np.where(
            in1 > (in0 * s1 + s0),
            (in0 * imm2).astype(np.float32) + np.float32(1.0),
            (in0 * imm2).astype(np.float32),
        ).astype(np.float32),
    ),
)

# s_t = (a_{t+1} != la*a_t);  in0=a[t+1] stream, in1=a[t] stream
ALIF_SRECOV = _register(
    "ALIF_SRECOV",
    Spec(
        body=ne(Src0, Src1 * C0),
        reference=lambda in0, in1, s0, s1, imm2: (
            in0 != (in1 * s0).astype(np.float32)
        ).astype(np.float32),
    ),
)


# ------------------------------------------------------------- graph build
def build_graph(lam_v: float, lam_a: float, theta: float, beta: float) -> bass.Bass:
    f32 = mybir.dt.float32
    nc = bacc.Bacc(
        "TRN2",
        target_bir_lowering=False,
        debug=False,
        num_devices=N_CORES,
    )

    x_ext = nc.dram_tensor("x", [T, NCORE], f32, kind="ExternalInput").ap()
    v_ext = nc.dram_tensor("v", [P, F], f32, kind="ExternalInput").ap()
    a_ext = nc.dram_tensor("a", [P, F], f32, kind="ExternalInput").ap()
    out_ext = nc.dram_tensor("out", [T, NCORE], f32, kind="ExternalOutput").ap()

    with TileContext(nc) as tc:
        with (
            tc.tile_pool(name="state", bufs=1) as state_pool,
            tc.tile_pool(name="xs", bufs=3) as x_pool,
            tc.tile_pool(name="ss", bufs=3) as s_pool,
            tc.tile_pool(name="at", bufs=2) as a_pool,
            tc.tile_pool(name="wt", bufs=2) as w_pool,
        ):
            v = state_pool.tile([P, F], f32)
            nc.sync.dma_start(out=v[:], in_=v_ext[:])

            prev_atraj = None
            for k in range(NCHUNK):
                xt = x_pool.tile([P, FD], f32, tag="x")
                nc.sync.dma_start(
                    out=xt[:].rearrange("p (t f) -> p t f", f=F),
                    in_=x_ext[k * TC : (k + 1) * TC, :].rearrange(
                        "t (p f) -> p t f", p=P
                    ),
                )

                atraj = a_pool.tile([P, (TC + 1) * F], f32, tag="atraj")
                if k == 0:
                    nc.sync.dma_start(out=atraj[:, 0:F], in_=a_ext[:])
                else:
                    nc.vector.tensor_copy(
                        atraj[:, 0:F], prev_atraj[:, TC * F : (TC + 1) * F]
                    )

                for t in range(TC):
                    w = w_pool.tile([P, F], f32, tag="w")
                    a_prev = atraj[:, t * F : (t + 1) * F]
                    x_t = xt[:, t * F : (t + 1) * F]
                    nc.vector._custom_dve(
                        AFFINE_THEN_ADD, out=w[:], in0=v[:], in1=x_t,
                        s0=lam_v, s1=0.0,
                    )
                    nc.vector._custom_dve(
                        ALIF_ASPIKE,
                        out=atraj[:, (t + 1) * F : (t + 2) * F],
                        in0=a_prev, in1=w[:],
                        s0=theta, s1=beta, imm2=lam_a,
                    )
                    nc.vector._custom_dve(
                        ALIF_RESET, out=v[:], in0=w[:], in1=a_prev,
                        s0=theta, s1=beta,
                    )

                st = s_pool.tile([P, FD], f32, tag="s")
                nc.vector._custom_dve(
                    ALIF_SRECOV,
                    out=st[:],
                    in0=atraj[:, F : (TC + 1) * F],
                    in1=atraj[:, 0 : TC * F],
                    s0=lam_a,
                )
                nc.sync.dma_start(
                    out=out_ext[k * TC : (k + 1) * TC, :].rearrange(
                        "t (p f) -> p t f", p=P
                    ),
                    in_=st[:].rearrange("p (t f) -> p t f", f=F),
                )
                prev_atraj = atraj

    nc.compile()
    return nc


_GRAPH_CACHE: dict[tuple, bass.Bass] = {}


def _get_graph(lam_v, lam_a, theta, beta):
    key = (lam_v, lam_a, theta, beta)
    if key not in _GRAPH_CACHE:
        _GRAPH_CACHE[key] = build_graph(lam_v, lam_a, theta, beta)
    return _GRAPH_CACHE[key]


# ------------------------------------------------------------------ entry
def kernel(
    x_seq, v, a, decay_v, decay_a, threshold, beta, alpha, _want_trace=False
):
    x = np.ascontiguousarray(np.asarray(x_seq, dtype=np.float32))
    orig_shape = x.shape
    x2 = x.reshape(T, N_FLAT)
    v = np.asarray(v, dtype=np.float32).reshape(N_FLAT)
    a = np.asarray(a, dtype=np.float32).reshape(N_FLAT)

    lam_v = float(np.float32(decay_v))
    lam_a = float(np.float32(decay_a))
    theta = float(np.float32(threshold))
    beta_f = float(np.float32(beta))

    nc = _get_graph(lam_v, lam_a, theta, beta_f)

    in_maps = []
    for c in range(N_CORES):
        lo, hi = c * NCORE, (c + 1) * NCORE
        in_maps.append(
            {
                "x": np.ascontiguousarray(x2[:, lo:hi]),
                "v": np.ascontiguousarray(v[lo:hi].reshape(P, F)),
                "a": np.ascontiguousarray(a[lo:hi].reshape(P, F)),
            }
        )

    res = run_bass_kernel_spmd(
        nc, in_maps, core_ids=list(range(N_CORES)), trace=_want_trace
    )
    outs = [res.results[c]["out"] for c in range(N_CORES)]
    full = np.concatenate(outs, axis=1).reshape(orig_shape)
    if _want_trace:
        return full, res
    return full
